# revision 1
# baseline (speedup 1.0000x reference)
"""KNNGraph (k=16) Bass kernel for 8 NeuronCores.

Input: x (4, 8192, 64) fp32. Output: (src, dst) int32 edge arrays of the
16-NN graph per batch (self included), matching jax.lax.top_k(-d2) order.

Sharding: core c handles batch c//2, query rows (c%2)*4096 ... +4096,
against all 8192 keys of that batch (query-row sharding, keys replicated).

Per core: for each of 32 groups of 128 query rows
  PE:  16 fp32 matmuls (K=65: 64 dims + ones row folding -|key|^2/2)
       -> PSUM chunks (128, 512) of w = q.k - |k|^2/2  (rank-equiv to -d2/2)
  ACT: copy PSUM -> SBUF w row buffer (128, 8192)
  DVE: per-chunk top-8 (max) -> 128 candidates; merge to top-16 values
       (max / match_replace / max); global indices via 2x max_index on w.
Host: assembles edges; verifies per-row sufficiency (8th-of-chunk >= 16th
overall => rescan needed) and recomputes rare ambiguous rows exactly.
"""

import numpy as np

N, M, D = 4, 8192, 64
K = 16
NCORES = 8
QROWS = M // 2          # query rows per core
NGROUPS = QROWS // 128  # 32
NCHUNK = 16             # key chunks of 512
CHUNK = M // NCHUNK     # 512
KDIM = 128              # contraction rows (64 dims + 1 ones + zero pad)

_COMPILED = {}


def _build_nc():
    import concourse.bacc as bacc
    import concourse.mybir as mybir
    import concourse.tile as tile

    nc = bacc.Bacc(None)
    f32 = mybir.dt.float32
    u32 = mybir.dt.uint32

    q_d = nc.declare_dram_parameter("q", [KDIM, QROWS], f32, isOutput=False)
    kv_d = nc.declare_dram_parameter("kv", [KDIM, M], f32, isOutput=False)
    idx_d = nc.declare_dram_parameter("idx", [NGROUPS, 128, K], u32, isOutput=True)
    val_d = nc.declare_dram_parameter("val", [NGROUPS, 128, K], f32, isOutput=True)
    c8_d = nc.declare_dram_parameter("c8", [NGROUPS, 128, NCHUNK * 8], f32, isOutput=True)

    with tile.TileContext(nc) as tc:
        with (
            tc.tile_pool(name="singles", bufs=1) as singles,
            tc.tile_pool(name="wbuf", bufs=2) as wpool,
            tc.tile_pool(name="psum", bufs=8, space="PSUM") as psum,
            tc.tile_pool(name="cands", bufs=2) as cands,
            tc.tile_pool(name="smalls", bufs=2) as smalls,
        ):
            q_sb = singles.tile([KDIM, QROWS], f32)
            kv_sb = singles.tile([KDIM, M], f32)
            nc.gpsimd.dma_start(out=q_sb[:], in_=q_d[:])
            nc.gpsimd.dma_start(out=kv_sb[:], in_=kv_d[:])

            for g in range(NGROUPS):
                w = wpool.tile([128, M], f32, tag="w")
                c8 = cands.tile([128, NCHUNK * 8], f32, tag="c8")
                lhsT = q_sb[:, g * 128:(g + 1) * 128]
                for c in range(NCHUNK):
                    pt = psum.tile([128, CHUNK], f32, tag="pt")
                    nc.tensor.matmul(
                        pt[:], lhsT, kv_sb[:, c * CHUNK:(c + 1) * CHUNK],
                        start=True, stop=True,
                    )
                    nc.scalar.copy(out=w[:, c * CHUNK:(c + 1) * CHUNK], in_=pt[:])
                    nc.vector.max(
                        out=c8[:, c * 8:(c + 1) * 8],
                        in_=w[:, c * CHUNK:(c + 1) * CHUNK],
                    )
                v8a = smalls.tile([128, 8], f32, tag="v8a")
                v8b = smalls.tile([128, 8], f32, tag="v8b")
                c8m = smalls.tile([128, NCHUNK * 8], f32, tag="c8m")
                i8a = smalls.tile([128, 8], u32, tag="i8a")
                i8b = smalls.tile([128, 8], u32, tag="i8b")
                nc.vector.max(out=v8a[:], in_=c8[:])
                nc.vector.match_replace(
                    out=c8m[:], in_to_replace=v8a[:], in_values=c8[:],
                    imm_value=-3.0e38,
                )
                nc.vector.max(out=v8b[:], in_=c8m[:])
                nc.vector.max_index(out=i8a[:], in_max=v8a[:], in_values=w[:])
                nc.vector.max_index(out=i8b[:], in_max=v8b[:], in_values=w[:])

                nc.sync.dma_start(out=idx_d[g, :, 0:8], in_=i8a[:])
                nc.sync.dma_start(out=idx_d[g, :, 8:16], in_=i8b[:])
                nc.sync.dma_start(out=val_d[g, :, 0:8], in_=v8a[:])
                nc.sync.dma_start(out=val_d[g, :, 8:16], in_=v8b[:])
                nc.sync.dma_start(out=c8_d[g], in_=c8[:])
    if not nc.is_finalized():
        nc.finalize()
    return nc


def _prep_inputs(x):
    """Per-core input dicts. x: (N, M, D) fp32."""
    x64 = x.astype(np.float64)
    x2 = (x64 * x64).sum(-1)          # (N, M) exact-ish
    neg_half_x2 = (-0.5 * x2).astype(np.float32)
    in_maps = []
    for c in range(NCORES):
        b, h = c // 2, c % 2
        q = np.zeros((KDIM, QROWS), np.float32)
        q[:D] = x[b, h * QROWS:(h + 1) * QROWS, :].T
        q[D] = 1.0
        kv = np.zeros((KDIM, M), np.float32)
        kv[:D] = x[b].T
        kv[D] = neg_half_x2[b]
        in_maps.append({"q": q, "kv": kv})
    return in_maps


def _host_topk_row(x64, b, r):
    """Exact fp64 top-K for one row; returns (idx, order ascending d2)."""
    d2 = ((x64[b] - x64[b, r]) ** 2).sum(-1)
    part = np.argpartition(d2, K)[:K]
    order = part[np.argsort(d2[part], kind="stable")]
    return order


def kernel(x, k):
    x = np.asarray(x, dtype=np.float32)
    k = int(k)
    assert x.shape == (N, M, D) and k == K

    from concourse.bass_utils import run_bass_kernel_spmd

    if "nc" not in _COMPILED:
        _COMPILED["nc"] = _build_nc()
    nc = _COMPILED["nc"]

    in_maps = _prep_inputs(x)
    res = run_bass_kernel_spmd(nc, in_maps, list(range(NCORES))).results

    idx = np.empty((N, M, K), np.int64)
    val = np.empty((N, M, K), np.float64)
    c8 = np.empty((N, M, NCHUNK * 8), np.float64)
    for c in range(NCORES):
        b, h = c // 2, c % 2
        sl = slice(h * QROWS, (h + 1) * QROWS)
        idx[b, sl] = res[c]["idx"].reshape(QROWS, K)
        val[b, sl] = res[c]["val"].reshape(QROWS, K)
        c8[b, sl] = res[c]["c8"].reshape(QROWS, NCHUNK * 8)

    # ---- host verification / rare-row fallback -------------------------
    x64 = x.astype(np.float64)
    t16 = val[..., K - 1]                      # 16th-largest w
    m8 = c8[..., 7::8]                         # (N, M, 16) 8th of each chunk
    suspect = (m8 >= t16[..., None]).any(-1)
    # duplicate indices or non-strictly-descending values
    sv = np.sort(idx, axis=-1)
    suspect |= (sv[..., 1:] == sv[..., :-1]).any(-1)
    suspect |= (np.diff(val, axis=-1) >= 0).any(-1)
    nbad = int(suspect.sum())
    if nbad:
        for b, r in zip(*np.nonzero(suspect)):
            idx[b, r] = _host_topk_row(x64, b, r)

    offset = (np.arange(N, dtype=np.int64) * M)[:, None, None]
    src = (idx + offset).reshape(-1).astype(np.int32)
    dst = np.repeat(np.arange(N * M, dtype=np.int32), K)
    return src, dst


if __name__ == "__main__":
    rng = np.random.default_rng(0)
    xt = rng.standard_normal((N, M, D), dtype=np.float32)
    s, d = kernel(xt, 16)
    print(s[:32], d[:32])

